# revision 25
# baseline (speedup 1.0000x reference)
"""CapsuleLoss Trainium2 kernel.

Data-parallel over batch B=8 across 8 NeuronCores (one image per core).

The environment's jax lowers the reference's scatter-``max`` as
scatter-``add``, so the oracle semantics are per-class *pixel counts*:

    image_labels[b,c] = #{valid pixels of image b with label c}
    loss_b = sum_c [ cnt_c*left_c + 0.5*(1-cnt_c)*right_c ]
           = sum_c cnt_c * d_c + 0.5*sum_c right_c,   d_c = left_c - 0.5*right_c

Per core we need 21 per-class counts of a [512,512] int32 label map,
obtained with fused compare+accumulate reduction passes split across
two engines (this toolchain's Pool engine only supports memset/copy/DMA):
  - DVE : tensor_scalar(is_equal c, accum add) in bf16 (4x mode) for
          classes 0..KEQ-1, on m = bf16(labels) (values {0..20,255},
          all exact in bf16; 255 never equals a class).
  - ACT : activation(Sign, bias=0.5-c, accum_out) on the RAW int32
          labels for c in {KEQ..21}: S_c = sum sign(l-c+0.5) = 2*T_c - N
          where T_c = #{l >= c}.  n_c = T_c - T_{c+1} = (S_c - S_{c+1})/2
          for c <= 20; the ignore pixels (255) cancel in the difference.
          All sums are integer-exact in fp32.

Each pass writes its per-partition partial sums into a distinct column
of a [128,32] fp32 accumulator tile (per engine per chunk).  The tiles
are merged and collapsed over partitions with a single PE matmul against
a ones-vector (PSUM [1,32]); a device-built weight row (d_c for eq
classes, the (D_c - D_{c-1})/2 stencil for sign columns, plus a
0.5*right column) turns it into loss_b = sum(S_row * W_row).  The host
averages the 8 per-core losses (the final mean "all-reduce").
"""

import numpy as np

NUM_CLASSES = 21
M_PLUS = 0.9
M_MINUS = 0.1
LAMBDA = 0.5

P = 128            # SBUF partitions
FTOT = 2048        # free elems per partition (128*2048 = 512*512)
NCH = 2            # DMA/compute chunks
CF = FTOT // NCH   # elems per partition per chunk

KEQ = 17                                      # classes counted on DVE (0..KEQ-1)
SIGN_C = list(range(KEQ, NUM_CLASSES + 1))    # ACT sign-pass offsets c (KEQ..21)
# accumulator column layout ([128,32] tiles, column == S row)
#   cols 0..KEQ-1                 : eq-class counts (DVE)
#   cols KEQ..KEQ+len(SIGN_C)-1   : sign sums S_c (ACT)
#   col 23                        : 0.5*right_c (margin math, partitions 0..20)
RHALF_COL = 23


def build_nc():
    """Build the single-core Bass module (same program for all 8 cores)."""
    from concourse import mybir
    from concourse.bacc import Bacc
    from concourse.mybir import AluOpType as OP
    from concourse.tile import TileContext

    AF = mybir.ActivationFunctionType
    f32 = mybir.dt.float32
    i32 = mybir.dt.int32
    bf16 = mybir.dt.bfloat16

    nc = Bacc(trn_type="TRN2")
    labels_d = nc.dram_tensor("labels", (512, 512), i32, kind="ExternalInput")
    caps_d = nc.dram_tensor("caps", (NUM_CLASSES, 16), f32, kind="ExternalInput")
    loss_d = nc.dram_tensor("loss", (1, 1), f32, kind="ExternalOutput")

    lab_flat = labels_d[:].flatten()

    with TileContext(nc) as tc:
        with tc.tile_pool(name="labs", bufs=NCH + 1) as lpool, \
             tc.tile_pool(name="work", bufs=NCH + 1) as wpool, \
             tc.tile_pool(name="small", bufs=1) as spool:

            # ---- accumulator tiles (one per engine per chunk), zeroed ----
            acc_d = [spool.tile([P, 32], f32, name=f"acc_d{c}") for c in range(NCH)]
            acc_a = [spool.tile([P, 32], f32, name=f"acc_a{c}") for c in range(NCH)]
            for t in acc_d + acc_a:
                nc.vector.memset(t[:], 0.0)

            # ---- capsule lengths + margin pieces (ACT + small DVE ops) ----
            caps_t = spool.tile([NUM_CLASSES, 16], f32)
            nc.sync.dma_start(out=caps_t[:], in_=caps_d[:])
            sq = spool.tile([NUM_CLASSES, 16], f32)
            len2 = spool.tile([NUM_CLASSES, 1], f32)
            nc.scalar.activation(sq[:], caps_t[:], AF.Square, accum_out=len2[:])
            lens = spool.tile([NUM_CLASSES, 1], f32)
            nc.scalar.activation(lens[:], len2[:], AF.Sqrt)

            # a = min(len-0.9, 0) -> a*a == relu(0.9-len)^2
            a = spool.tile([NUM_CLASSES, 1], f32)
            nc.vector.tensor_scalar(a[:], lens[:], M_PLUS, 0.0, OP.subtract, OP.min)
            # b = max(len-0.1, 0) -> b*b == relu(len-0.1)^2
            b = spool.tile([NUM_CLASSES, 1], f32)
            nc.vector.tensor_scalar(b[:], lens[:], M_MINUS, 0.0, OP.subtract, OP.max)
            left = spool.tile([NUM_CLASSES, 1], f32)
            nc.vector.tensor_tensor(left[:], a[:], a[:], OP.mult)
            # 0.5*right: into its own tile and into accumulator col 23
            rhalf = spool.tile([NUM_CLASSES, 1], f32)
            nc.vector.scalar_tensor_tensor(rhalf[:], b[:], LAMBDA, b[:],
                                           OP.mult, OP.mult)
            nc.vector.tensor_copy(acc_d[0][0:NUM_CLASSES, RHALF_COL:RHALF_COL + 1],
                                  rhalf[:])

            # ---- weight row W (free layout; partition starts must be
            #      0/32/64/96, so shifted stencils use free-dim slices) ----
            # transpose left (block 0) and rhalf (block 1) into row 0
            LR = spool.tile([32, 64], f32)
            nc.vector.memset(LR[:], 0.0)
            nc.vector.tensor_copy(LR[0:NUM_CLASSES, 0:1], left[:])
            nc.vector.tensor_copy(LR[0:NUM_CLASSES, 32:33], rhalf[:])
            LRT = spool.tile([32, 64], f32)
            nc.vector.transpose(LRT[:], LR[:])
            # d_row = left - 0.5*right  (cols = classes 0..20; rest 0)
            dR = spool.tile([1, 32], f32)
            nc.vector.tensor_tensor(dR[:], LRT[0:1, 0:32], LRT[0:1, 32:64],
                                    OP.subtract)
            # D_row: d/2 masked to sign-recovered classes {KEQ..20}
            Drow = spool.tile([1, 32], f32)
            nc.vector.memset(Drow[:], 0.0)
            nc.vector.tensor_scalar(Drow[0:1, KEQ:NUM_CLASSES],
                                    dR[0:1, KEQ:NUM_CLASSES], 0.5, None,
                                    OP.mult)
            # W_row: d_c for eq classes; (D_c - D_{c-1}) for sign cols
            Wrow = spool.tile([1, 32], f32)
            nc.vector.memset(Wrow[:], 0.0)
            nc.vector.tensor_copy(Wrow[0:1, 0:KEQ], dR[0:1, 0:KEQ])
            ns = len(SIGN_C)
            c0 = SIGN_C[0]
            nc.vector.tensor_tensor(
                Wrow[0:1, KEQ:KEQ + ns], Drow[0:1, c0:c0 + ns],
                Drow[0:1, c0 - 1:c0 - 1 + ns], OP.subtract)
            nc.vector.memset(Wrow[0:1, RHALF_COL:RHALF_COL + 1], 1.0)

            # per-partition bias columns for the ACT sign passes (0.5 - c)
            bias_t = spool.tile([P, ns], f32)
            for i, c in enumerate(SIGN_C):
                nc.vector.memset(bias_t[:, i:i + 1], 0.5 - float(c))

            # ---- main label scan ----
            for ci in range(NCH):
                lt = lpool.tile([P, CF], i32, tag="lab")
                chunk = lab_flat[ci * P * CF:(ci + 1) * P * CF] \
                    .rearrange("(p f) -> p f", p=P)
                nc.sync.dma_start(out=lt[:], in_=chunk)

                # bf16 copy of the labels for the DVE 4x eq passes
                m = wpool.tile([P, CF], bf16, tag="m")
                nc.vector.tensor_copy(m[:], lt[:])

                junk_d = wpool.tile([P, CF], bf16, tag="junk_d")
                junk_a = wpool.tile([P, CF], bf16, tag="junk_a")

                # DVE equality counts on bf16 (4x mode)
                for c in range(KEQ):
                    nc.vector.tensor_scalar(
                        junk_d[:], m[:], float(c), None, OP.is_equal, OP.add,
                        accum_out=acc_d[ci][:, c:c + 1])
                # ACT sign passes on the raw int32 labels
                for i, c in enumerate(SIGN_C):
                    nc.scalar.activation(
                        junk_a[:], lt[:], AF.Sign, bias=bias_t[:, i:i + 1],
                        scale=1.0,
                        accum_out=acc_a[ci][:, KEQ + i:KEQ + i + 1])

            # ---- merge accumulators, collapse partitions via PE matmul ----
            M = acc_d[0]
            for t in acc_d[1:] + acc_a:
                nc.vector.tensor_tensor(M[:], M[:], t[:], OP.add)
            ones128 = spool.tile([P, 1], f32)
            nc.vector.memset(ones128[:], 1.0)
            with tc.tile_pool(name="ps", bufs=1, space="PSUM") as pspool:
                Srow = pspool.tile([1, 32], f32)
                nc.tensor.matmul(Srow[:], ones128[:], M[:], start=True,
                                 stop=True)
                # loss = sum(S_row * W_row)
                e = spool.tile([1, 32], f32)
                nc.vector.tensor_tensor(e[:], Srow[:], Wrow[:], OP.mult)
            loss_t = spool.tile([1, 1], f32)
            nc.vector.tensor_reduce(loss_t[:], e[:], mybir.AxisListType.X,
                                    OP.add)
            nc.sync.dma_start(out=loss_d[:], in_=loss_t[:])

    nc.finalize()
    return nc


_NC_CACHE = None


def _get_nc():
    global _NC_CACHE
    if _NC_CACHE is None:
        _NC_CACHE = build_nc()
    return _NC_CACHE


def make_in_maps(class_caps, labels):
    return [
        {
            "labels": np.ascontiguousarray(labels[b], dtype=np.int32),
            "caps": np.ascontiguousarray(class_caps[b], dtype=np.float32),
        }
        for b in range(8)
    ]


def kernel(class_caps, labels, trace=False):
    from concourse.bass_utils import run_bass_kernel_spmd

    class_caps = np.asarray(class_caps)
    labels = np.asarray(labels)
    nc = _get_nc()
    res = run_bass_kernel_spmd(nc, make_in_maps(class_caps, labels),
                               core_ids=list(range(8)), trace=trace)
    losses = np.array([r["loss"][0, 0] for r in res.results], dtype=np.float32)
    out = np.float32(losses.mean())
    if trace:
        return out, res
    return out


# revision 27
# speedup vs baseline: 1.2307x; 1.2307x over previous
"""CapsuleLoss Trainium2 kernel.

Data-parallel over batch B=8 across 8 NeuronCores (one image per core).

The environment's jax lowers the reference's scatter-``max`` as
scatter-``add``, so the oracle semantics are per-class *pixel counts*:

    image_labels[b,c] = #{valid pixels of image b with label c}
    loss_b = sum_c [ cnt_c*left_c + 0.5*(1-cnt_c)*right_c ]
           = sum_c cnt_c * d_c + 0.5*sum_c right_c,   d_c = left_c - 0.5*right_c

Per core we need 21 per-class counts of a [512,512] int32 label map,
obtained with fused compare+accumulate reduction passes split across
two engines (this toolchain's Pool engine only supports memset/copy/DMA):
  - DVE : tensor_scalar(is_equal c, accum add) on a bf16 copy of the
          labels for classes 0..KEQ-1 (the accumulate-fused TS uop runs
          at 1x and rejects int32 input; 255 never equals a class).
  - ACT : activation(Sign, bias=0.5-c, accum_out) on the RAW int32
          labels for c in {KEQ..21}: S_c = sum sign(l-c+0.5) = 2*T_c - N
          where T_c = #{l >= c}.  n_c = T_c - T_{c+1} = (S_c - S_{c+1})/2
          for c <= 20; the ignore pixels (255) cancel in the difference.
          All sums are integer-exact in fp32.

Each pass writes its per-partition partial sums into a distinct column
of a [128,32] fp32 accumulator tile (per engine per chunk).  The tiles
are merged and collapsed over partitions with a single PE matmul against
a ones-vector (PSUM [1,32]); a device-built weight row (d_c for eq
classes, the (D_c - D_{c-1})/2 stencil for sign columns, plus a
0.5*right column) turns it into loss_b = sum(S_row * W_row).  The host
averages the 8 per-core losses (the final mean "all-reduce").
"""

import numpy as np

NUM_CLASSES = 21
M_PLUS = 0.9
M_MINUS = 0.1
LAMBDA = 0.5

P = 128            # SBUF partitions
FTOT = 2048        # free elems per partition (128*2048 = 512*512)
NCH = 2            # DMA/compute chunks
CF = FTOT // NCH   # elems per partition per chunk

KEQ = 12                                      # classes counted on DVE (0..KEQ-1)
SIGN_C = list(range(KEQ, NUM_CLASSES + 1))    # ACT sign-pass offsets c (KEQ..21)
# accumulator column layout ([128,32] tiles, column == S row)
#   cols 0..KEQ-1                 : eq-class counts (DVE)
#   cols KEQ..KEQ+len(SIGN_C)-1   : sign sums S_c (ACT)
#   col 23                        : 0.5*right_c (margin math, partitions 0..20)
RHALF_COL = 23


def build_nc():
    """Build the single-core Bass module (same program for all 8 cores)."""
    from concourse import mybir
    from concourse.bacc import Bacc
    from concourse.mybir import AluOpType as OP
    from concourse.tile import TileContext

    AF = mybir.ActivationFunctionType
    f32 = mybir.dt.float32
    i32 = mybir.dt.int32
    bf16 = mybir.dt.bfloat16

    nc = Bacc(trn_type="TRN2")
    labels_d = nc.dram_tensor("labels", (512, 512), i32, kind="ExternalInput")
    caps_d = nc.dram_tensor("caps", (NUM_CLASSES, 16), f32, kind="ExternalInput")
    loss_d = nc.dram_tensor("loss", (1, 1), f32, kind="ExternalOutput")

    lab_flat = labels_d[:].flatten()

    with TileContext(nc) as tc:
        with tc.tile_pool(name="labs", bufs=NCH + 1) as lpool, \
             tc.tile_pool(name="work", bufs=NCH + 1) as wpool, \
             tc.tile_pool(name="small", bufs=1) as spool:

            # ---- accumulator tiles (one per engine per chunk), zeroed ----
            acc_d = [spool.tile([P, 32], f32, name=f"acc_d{c}") for c in range(NCH)]
            acc_a = [spool.tile([P, 32], f32, name=f"acc_a{c}") for c in range(NCH)]
            for t in acc_d + acc_a:
                nc.vector.memset(t[:], 0.0)

            # ---- capsule lengths + margin pieces (ACT + small DVE ops) ----
            caps_t = spool.tile([NUM_CLASSES, 16], f32)
            nc.sync.dma_start(out=caps_t[:], in_=caps_d[:])
            sq = spool.tile([NUM_CLASSES, 16], f32)
            len2 = spool.tile([NUM_CLASSES, 1], f32)
            nc.scalar.activation(sq[:], caps_t[:], AF.Square, accum_out=len2[:])
            lens = spool.tile([NUM_CLASSES, 1], f32)
            nc.scalar.activation(lens[:], len2[:], AF.Sqrt)

            # a = min(len-0.9, 0) -> a*a == relu(0.9-len)^2
            a = spool.tile([NUM_CLASSES, 1], f32)
            nc.vector.tensor_scalar(a[:], lens[:], M_PLUS, 0.0, OP.subtract, OP.min)
            # b = max(len-0.1, 0) -> b*b == relu(len-0.1)^2
            b = spool.tile([NUM_CLASSES, 1], f32)
            nc.vector.tensor_scalar(b[:], lens[:], M_MINUS, 0.0, OP.subtract, OP.max)
            left = spool.tile([NUM_CLASSES, 1], f32)
            nc.vector.tensor_tensor(left[:], a[:], a[:], OP.mult)
            # 0.5*right: into its own tile and into accumulator col 23
            rhalf = spool.tile([NUM_CLASSES, 1], f32)
            nc.vector.scalar_tensor_tensor(rhalf[:], b[:], LAMBDA, b[:],
                                           OP.mult, OP.mult)
            nc.vector.tensor_copy(acc_d[0][0:NUM_CLASSES, RHALF_COL:RHALF_COL + 1],
                                  rhalf[:])

            # ---- weight row W (free layout; partition starts must be
            #      0/32/64/96, so shifted stencils use free-dim slices) ----
            # transpose left (block 0) and rhalf (block 1) into row 0
            LR = spool.tile([32, 64], f32)
            nc.vector.memset(LR[:], 0.0)
            nc.vector.tensor_copy(LR[0:NUM_CLASSES, 0:1], left[:])
            nc.vector.tensor_copy(LR[0:NUM_CLASSES, 32:33], rhalf[:])
            LRT = spool.tile([32, 64], f32)
            nc.vector.transpose(LRT[:], LR[:])
            # d_row = left - 0.5*right  (cols = classes 0..20; rest 0)
            dR = spool.tile([1, 32], f32)
            nc.vector.tensor_tensor(dR[:], LRT[0:1, 0:32], LRT[0:1, 32:64],
                                    OP.subtract)
            # D_row: d/2 masked to sign-recovered classes {KEQ..20}
            Drow = spool.tile([1, 32], f32)
            nc.vector.memset(Drow[:], 0.0)
            nc.vector.tensor_scalar(Drow[0:1, KEQ:NUM_CLASSES],
                                    dR[0:1, KEQ:NUM_CLASSES], 0.5, None,
                                    OP.mult)
            # W_row: d_c for eq classes; (D_c - D_{c-1}) for sign cols
            Wrow = spool.tile([1, 32], f32)
            nc.vector.memset(Wrow[:], 0.0)
            nc.vector.tensor_copy(Wrow[0:1, 0:KEQ], dR[0:1, 0:KEQ])
            ns = len(SIGN_C)
            c0 = SIGN_C[0]
            nc.vector.tensor_tensor(
                Wrow[0:1, KEQ:KEQ + ns], Drow[0:1, c0:c0 + ns],
                Drow[0:1, c0 - 1:c0 - 1 + ns], OP.subtract)
            nc.vector.memset(Wrow[0:1, RHALF_COL:RHALF_COL + 1], 1.0)

            # per-partition bias columns for the ACT sign passes (0.5 - c)
            bias_t = spool.tile([P, ns], f32)
            for i, c in enumerate(SIGN_C):
                nc.vector.memset(bias_t[:, i:i + 1], 0.5 - float(c))

            # ---- main label scan ----
            for ci in range(NCH):
                lt = lpool.tile([P, CF], i32, tag="lab")
                chunk = lab_flat[ci * P * CF:(ci + 1) * P * CF] \
                    .rearrange("(p f) -> p f", p=P)
                nc.sync.dma_start(out=lt[:], in_=chunk)

                # bf16 view of the labels: the accum-fused TS rejects int32
                # input (tensor_scalar_cache_reduce_valid_types); values
                # {0..20,255} are exact in bf16
                m = wpool.tile([P, CF], bf16, tag="m")
                nc.vector.tensor_copy(m[:], lt[:])

                junk_d = wpool.tile([P, CF], bf16, tag="junk_d")
                junk_a = wpool.tile([P, CF], bf16, tag="junk_a")

                # DVE equality counts (accum-fused TS runs at 1x)
                for c in range(KEQ):
                    nc.vector.tensor_scalar(
                        junk_d[:], m[:], float(c), None, OP.is_equal, OP.add,
                        accum_out=acc_d[ci][:, c:c + 1])
                # ACT sign passes on the raw int32 labels
                for i, c in enumerate(SIGN_C):
                    nc.scalar.activation(
                        junk_a[:], lt[:], AF.Sign, bias=bias_t[:, i:i + 1],
                        scale=1.0,
                        accum_out=acc_a[ci][:, KEQ + i:KEQ + i + 1])

            # ---- merge accumulators, collapse partitions via PE matmul ----
            M = acc_d[0]
            for t in acc_d[1:] + acc_a:
                nc.vector.tensor_tensor(M[:], M[:], t[:], OP.add)
            ones128 = spool.tile([P, 1], f32)
            nc.vector.memset(ones128[:], 1.0)
            with tc.tile_pool(name="ps", bufs=1, space="PSUM") as pspool:
                Srow = pspool.tile([1, 32], f32)
                nc.tensor.matmul(Srow[:], ones128[:], M[:], start=True,
                                 stop=True)
                # loss = sum(S_row * W_row)
                e = spool.tile([1, 32], f32)
                nc.vector.tensor_tensor(e[:], Srow[:], Wrow[:], OP.mult)
            loss_t = spool.tile([1, 1], f32)
            nc.vector.tensor_reduce(loss_t[:], e[:], mybir.AxisListType.X,
                                    OP.add)
            nc.sync.dma_start(out=loss_d[:], in_=loss_t[:])

    nc.finalize()
    return nc


_NC_CACHE = None


def _get_nc():
    global _NC_CACHE
    if _NC_CACHE is None:
        _NC_CACHE = build_nc()
    return _NC_CACHE


def make_in_maps(class_caps, labels):
    return [
        {
            "labels": np.ascontiguousarray(labels[b], dtype=np.int32),
            "caps": np.ascontiguousarray(class_caps[b], dtype=np.float32),
        }
        for b in range(8)
    ]


def kernel(class_caps, labels, trace=False):
    from concourse.bass_utils import run_bass_kernel_spmd

    class_caps = np.asarray(class_caps)
    labels = np.asarray(labels)
    nc = _get_nc()
    res = run_bass_kernel_spmd(nc, make_in_maps(class_caps, labels),
                               core_ids=list(range(8)), trace=trace)
    losses = np.array([r["loss"][0, 0] for r in res.results], dtype=np.float32)
    out = np.float32(losses.mean())
    if trace:
        return out, res
    return out
